# revision 2
# baseline (speedup 1.0000x reference)
import os
import sys

sys.path.insert(0, "/opt/trn_rl_repo")

import numpy as np

import concourse.bass as bass
import concourse.tile as tile
from concourse import mybir
from concourse.alu_op_type import AluOpType
from concourse.bass_utils import run_bass_kernel_spmd
from concourse.masks import make_identity

import orjson


def _split_multiwait(d: dict) -> dict:
    cnt = 0
    for fn in d.get("functions", []):
        for blk in fn.get("blocks", []):
            insts = blk.get("instructions", [])
            if not any(
                len((i.get("sync_info") or {}).get("on_wait") or []) > 1
                for i in insts
            ):
                continue
            new = []
            for ins in insts:
                si = ins.get("sync_info") or {}
                waits = si.get("on_wait") or []
                if len(waits) > 1:
                    for w in waits[:-1]:
                        cnt += 1
                        nop = {
                            "engine": ins["engine"],
                            "ins": [],
                            "outs": [],
                            "name": f"I-waitsplit-{cnt}",
                            "opcode": "NoOp",
                            "sync_info": {"on_update": [], "on_wait": [w]},
                            "text_hint": "wait_split",
                        }
                        if "debug" in ins:
                            nop["debug"] = ins["debug"]
                        new.append(nop)
                    si["on_wait"] = [waits[-1]]
                new.append(ins)
            blk["instructions"] = new
    return d


_orig_to_json_bytes = bass.Bass.to_json_bytes


def _patched_to_json_bytes(self) -> bytes:
    return orjson.dumps(_split_multiwait(orjson.loads(_orig_to_json_bytes(self))))


bass.Bass.to_json_bytes = _patched_to_json_bytes

N_CORES = 8
B, C_IN, H, WW = 16, 256, 2000, 9
C_OUT = 128
B_LOC = B // N_CORES
G, D = 4, 32
SCALE = 1.0 / np.sqrt(32.0)
MASK_A = float(np.sqrt(245.25))
MASK_S = float(np.sqrt(200.0))
HC = 14
PC = HC * WW
SUPER_H = 56
F16 = mybir.dt.float16
F32 = mybir.dt.float32
F32R = mybir.dt.float32r


def _superchunks():
    out = []
    h0 = 0
    while h0 + SUPER_H <= H:
        out.append((h0, SUPER_H))
        h0 += SUPER_H
    if h0 < H:
        out.append((h0, H - h0))
    return out


def _chunks(hcnt):
    out = []
    c0 = 0
    while c0 < hcnt:
        out.append((c0, min(HC, hcnt - c0)))
        c0 += HC
    return out


def _host_consts():
    maskL = np.zeros((128, PC), dtype=np.float16)
    maskR = np.zeros((128, PC), dtype=np.float16)
    for g in range(G):
        r0 = 32 * g
        maskL[r0, :] = MASK_A
        maskR[r0, :] = -MASK_A
        for hb in range(HC):
            maskL[r0 + 1 + hb, hb * WW:(hb + 1) * WW] = MASK_S
            maskR[r0 + 1 + hb, hb * WW:(hb + 1) * WW] = MASK_S
    return maskL, maskR


def _build_kernel():
    nc = bass.Bass("TRN2")
    X = nc.declare_dram_parameter("X", [B_LOC, C_IN, H, WW], F32R, isOutput=False)
    WT = nc.declare_dram_parameter("WT", [C_IN, C_OUT], F32R, isOutput=False)
    BIAS = nc.declare_dram_parameter("BIAS", [C_OUT, 1], F32, isOutput=False)
    ML = nc.declare_dram_parameter("ML", [128, PC], F16, isOutput=False)
    MR = nc.declare_dram_parameter("MR", [128, PC], F16, isOutput=False)
    OUT = nc.declare_dram_parameter("OUT", [B_LOC, C_OUT, H, WW], F32, isOutput=True)

    from contextlib import ExitStack
    with tile.TileContext(nc) as tc, ExitStack() as ctx:
        sb1 = ctx.enter_context(tc.tile_pool(name="sb1", bufs=1))
        sbx = ctx.enter_context(tc.tile_pool(name="sbx", bufs=3))
        sbf = ctx.enter_context(tc.tile_pool(name="sbf", bufs=2))
        sbo = ctx.enter_context(tc.tile_pool(name="sbo", bufs=2))
        sba = ctx.enter_context(tc.tile_pool(name="sba", bufs=2))
        psf = ctx.enter_context(tc.tile_pool(name="psf", bufs=1, space="PSUM"))
        psx = ctx.enter_context(tc.tile_pool(name="psx", bufs=1, space="PSUM"))
        pss = ctx.enter_context(tc.tile_pool(name="pss", bufs=1, space="PSUM"))
        psa = ctx.enter_context(tc.tile_pool(name="psa", bufs=1, space="PSUM"))

        wt0 = sb1.tile([128, C_OUT], F32R, name="wt0")
        wt1 = sb1.tile([128, C_OUT], F32R, name="wt1")
        nc.gpsimd.dma_start(out=wt0, in_=WT[0:128, :])
        nc.gpsimd.dma_start(out=wt1, in_=WT[128:256, :])
        bias = sb1.tile([C_OUT, 1], F32, name="bias")
        nc.gpsimd.dma_start(out=bias, in_=BIAS[:])
        ml = sb1.tile([128, PC], F16, name="ml")
        mr = sb1.tile([128, PC], F16, name="mr")
        nc.gpsimd.dma_start(out=ml, in_=ML[:])
        nc.gpsimd.dma_start(out=mr, in_=MR[:])
        ident = sb1.tile([128, 128], F16, name="ident")
        make_identity(nc, ident)
        ones32 = sb1.tile([128, 32], F16, name="ones32")
        nc.vector.memset(ones32, 1.0)

        for bi in range(B_LOC):
            for (h0, hcnt) in _superchunks():
                n = hcnt * WW
                nch = _chunks(hcnt)
                xx = sbx.tile([128, 2 * SUPER_H * WW], F32R, tag="xx")
                nc.gpsimd.dma_start(
                    out=xx[:, :n],
                    in_=X[bi, 0:128, h0:h0 + hcnt, :].rearrange("c h w -> c (h w)"),
                )
                nc.gpsimd.dma_start(
                    out=xx[:, n:2 * n],
                    in_=X[bi, 128:256, h0:h0 + hcnt, :].rearrange("c h w -> c (h w)"),
                )
                pfeat = psf.tile([C_OUT, 512], F32, tag="pfeat")
                nc.tensor.matmul(pfeat[:, :n], wt0, xx[:, :n], start=True, stop=False)
                nc.tensor.matmul(pfeat[:, :n], wt1, xx[:, n:2 * n], start=False, stop=True)
                feat = sbf.tile([C_OUT, SUPER_H * WW], F16, tag="feat")
                nc.vector.tensor_scalar(
                    out=feat[:, :n], in0=pfeat[:, :n],
                    scalar1=bias, scalar2=None, op0=AluOpType.add,
                )

                ncols = 128 * len(nch)
                if n < SUPER_H * WW:
                    nc.vector.memset(feat[:, n:], 0.0)
                xT = psx.tile([128, 512], F16, tag="xT")
                for ci, (c0, hc) in enumerate(nch):
                    nc.tensor.transpose(
                        xT[:PC, 128 * ci:128 * ci + 128],
                        feat[:, c0 * WW:c0 * WW + PC], ident,
                    )
                texp = sba.tile([PC, 512], F16, tag="texp")
                nc.scalar.activation(out=texp[:, :ncols], in_=xT[:PC, :ncols],
                                     func=mybir.ActivationFunctionType.Exp)
                u1 = sba.tile([PC, 512], F16, tag="u1")
                nc.vector.tensor_scalar(out=u1[:, :ncols], in0=texp[:, :ncols],
                                        scalar1=1.0, scalar2=-1.0,
                                        op0=AluOpType.min, op1=AluOpType.add)
                val = sba.tile([PC, 512], F16, tag="val")
                nc.vector.scalar_tensor_tensor(out=val[:, :ncols], in0=xT[:PC, :ncols],
                                               scalar=0.0, in1=u1[:, :ncols],
                                               op0=AluOpType.max, op1=AluOpType.add)

                ealls = []
                for ci, (c0, hc) in enumerate(nch):
                    p = hc * WW
                    fch = feat[:, c0 * WW:c0 * WW + p]
                    ssc = pss.tile([128, 2048], F32, tag="ssc", name=f"ssc{ci}")
                    for g in range(G):
                        r0 = 32 * g
                        col = 512 * g
                        nc.tensor.matmul(
                            ssc[:PC, col:col + PC],
                            ml[r0:r0 + 15, :], mr[r0:r0 + 15, :],
                            start=True, stop=False, tile_position=(r0, 0),
                        )
                        nc.tensor.matmul(
                            ssc[:p, col:col + p],
                            fch[r0:r0 + 32, :], fch[r0:r0 + 32, :],
                            start=False, stop=True, tile_position=(r0, 0),
                        )
                    eall = sba.tile([PC, 4 * PC], F16, tag="eall", name=f"eall{ci}")
                    nc.scalar.activation(
                        out=eall.rearrange("q (g c) -> q g c", g=4),
                        in_=ssc[:PC, :].rearrange("q (g c) -> q g c", g=4)[:, :, :PC],
                        func=mybir.ActivationFunctionType.Exp, scale=SCALE,
                    )
                    ealls.append(eall)

                av = psa.tile([C_OUT, 512], F32, tag="av")
                rr = psa.tile([C_OUT, 512], F32, tag="rr")
                for ci, (c0, hc) in enumerate(nch):
                    p = hc * WW
                    for g in range(G):
                        r0 = 32 * g
                        esl = ealls[ci][:p, PC * g:PC * g + p]
                        nc.tensor.matmul(
                            av[r0:r0 + 32, c0 * WW:c0 * WW + p],
                            val[:p, 128 * ci + r0:128 * ci + r0 + 32], esl,
                            start=True, stop=True, tile_position=(0, r0),
                        )
                        nc.tensor.matmul(
                            rr[r0:r0 + 32, c0 * WW:c0 * WW + p],
                            ones32[:p, :], esl,
                            start=True, stop=True, tile_position=(0, r0),
                        )
                rrr = sba.tile([C_OUT, SUPER_H * WW], F32, tag="rrr")
                nc.vector.reciprocal(out=rrr[:, :n], in_=rr[:, :n])
                o = sbo.tile([C_OUT, SUPER_H * WW], F32, tag="o")
                nc.vector.tensor_mul(o[:, :n], av[:, :n], rrr[:, :n])

                nc.gpsimd.dma_start(
                    out=OUT[bi, :, h0:h0 + hcnt, :].rearrange("c h w -> c (h w)"),
                    in_=o[:, :n],
                )

    return nc


_CACHED = {}


def _get_nc():
    if "nc" not in _CACHED:
        _CACHED["nc"] = _build_kernel()
    return _CACHED["nc"]


def _make_in_maps(inputs) -> list:
    x = np.asarray(inputs["neighbr_feats"], dtype=np.float32)
    w = np.asarray(inputs["W"], dtype=np.float32)
    b = np.asarray(inputs["b"], dtype=np.float32)

    wt = np.ascontiguousarray(w.T)
    bias = np.ascontiguousarray(b.reshape(C_OUT, 1))
    maskL, maskR = _host_consts()

    in_maps = []
    for core in range(N_CORES):
        xs = np.ascontiguousarray(x[core * B_LOC:(core + 1) * B_LOC])
        in_maps.append({
            "X": xs, "WT": wt, "BIAS": bias, "ML": maskL, "MR": maskR,
        })
    return in_maps


def kernel(**inputs) -> np.ndarray:
    nc = _get_nc()
    in_maps = _make_in_maps(inputs)
    res = run_bass_kernel_spmd(nc, in_maps, core_ids=list(range(N_CORES)))
    out = np.concatenate([r["OUT"] for r in res.results], axis=0)
    return out.astype(np.float32)


if __name__ == "__main__":
    rng = np.random.default_rng(0)
    inputs = {
        "neighbr_feats": rng.standard_normal((B, C_IN, H, WW)).astype(np.float32),
        "W": (rng.standard_normal((C_OUT, C_IN)) * 0.05).astype(np.float32),
        "b": (rng.standard_normal((C_OUT,)) * 0.05).astype(np.float32),
    }
    out = kernel(**inputs)
    print("kernel ran:", out.shape, out.dtype)

